# revision 20
# baseline (speedup 1.0000x reference)
"""Trainium2 Bass kernel for the DreamerV3-style ActorCriticLoss.

Contract: kernel(**inputs) takes the FULL (unsharded) numpy inputs and
returns the FULL output (a float32 scalar loss). Internally the batch dim
(B=4096) is sharded 8 ways (pure data parallel); each NeuronCore computes
everything except the two lambda-return quantiles and the final scalar
combine, which run on host after the gather (per-row work is all on
device; host only sums per-partition partials, takes the quantiles of the
device-computed lambda returns, and assembles the scalar).

Self-contained: hardcodes shapes from the problem spec.
"""

import sys
from contextlib import ExitStack

sys.path.insert(0, "/opt/trn_rl_repo")

import numpy as np

import concourse.bass as bass  # noqa: E402
import concourse.bacc as bacc  # noqa: E402
import concourse.mybir as mybir  # noqa: E402
from concourse import bass_utils  # noqa: E402
from concourse import tile  # noqa: E402

# ---- problem constants (from reference.py) ----
LOW, HIGH, NBINS = -20.0, 20.0, 255
GAMMA, LAM = 0.99, 0.95
ENT_COEF, SLOW_W = 0.05, 1.0
STEP = (HIGH - LOW) / (NBINS - 1)
B, T, A = 4096, 16, 32

NCORES = 8
BS = B // NCORES  # 512 batch rows per core
P = 128  # partitions
TB = BS // P  # 4 partition-blocks per core
NCOL = TB * T  # 64 columns in the assembled per-row tiles

F32 = mybir.dt.float32
I32 = mybir.dt.int32
Alu = mybir.AluOpType
Act = mybir.ActivationFunctionType

NEG_BIG = -3.0e38


def _ttr(nc, out, in0, in1, accum_out):
    """(in0*in1) elementwise with accum_out = sum — via the production
    custom-DVE op (the TENSOR_TENSOR_REDUCE ISA opcode crashes at runtime
    on this stack; the custom-DVE table path works)."""
    from concourse.dve_ops import TENSOR_TENSOR_REDUCE as _OP

    nc.vector._custom_dve(
        _OP, out=out, in0=in0, in1=in1, s0=0.0, s1=1.0, accum_out=accum_out
    )


def _mask_gather(nc, out, data, start_col, end_col, accum_out):
    """accum_out = max over idx in [start, end) of data — the width-1 range
    makes this an exact per-partition dynamic gather. Custom-DVE variant
    (the TENSOR_MASK_REDUCE ISA opcode crashes at runtime on this stack).
    C0=s0=start, C3=in1=end (spilled), C1=s1=accum seed, C2=imm2=scale."""
    from concourse.dve_ops import TENSOR_MASK_REDUCE as _OP

    nc.vector._custom_dve(
        _OP,
        out=out,
        in0=data,
        in1=end_col,
        s0=start_col,
        s1=NEG_BIG,
        imm2=1.0,
        accum_out=accum_out,
    )


def build_kernel(nc: bass.Bass, tc: "tile.TileContext", level: int = 99):
    """Per-core program. Inputs arrive with the T axis of `cont`/`actf`
    pre-reversed on host; for the big tensors we instead write per-(tb,t)
    results into column j = T-1-t, so all assembled [P, NCOL] tiles hold
    time-REVERSED columns. The lambda scan then runs forward along the
    free dim. lam_out column order is irrelevant on host (quantiles)."""

    # ---- DRAM I/O ----
    rew_d = nc.dram_tensor("rew", [BS, T, NBINS], F32, kind="ExternalInput").ap()
    slw_d = nc.dram_tensor("slw", [BS, T, NBINS], F32, kind="ExternalInput").ap()
    fst_d = nc.dram_tensor("fst", [BS, T, NBINS], F32, kind="ExternalInput").ap()
    actl_d = nc.dram_tensor("actl", [BS, T, A], F32, kind="ExternalInput").ap()
    # pre-reversed along T on host:
    cont_d = nc.dram_tensor("cont", [BS, T], F32, kind="ExternalInput").ap()
    actf_d = nc.dram_tensor("actf", [BS, T], F32, kind="ExternalInput").ap()

    lam_out = nc.dram_tensor("lam_out", [BS, T], F32, kind="ExternalOutput").ap()
    parts_out = nc.dram_tensor("parts_out", [P, 8], F32, kind="ExternalOutput").ap()

    rew_v = rew_d.rearrange("(tb p) t n -> tb p (t n)", p=P)
    slw_v = slw_d.rearrange("(tb p) t n -> tb p (t n)", p=P)
    fst_v = fst_d.rearrange("(tb p) t n -> tb p (t n)", p=P)
    actl_v = actl_d.rearrange("(tb p) t a -> tb p (t a)", p=P)
    cont_v = cont_d.rearrange("(tb p) t -> tb p t", p=P)
    actf_v = actf_d.rearrange("(tb p) t -> tb p t", p=P)
    lam_v = lam_out.rearrange("(tb p) t -> tb p t", p=P)

    ctx = ExitStack()
    const_pool = ctx.enter_context(tc.tile_pool(name="const", bufs=1))
    res_pool = ctx.enter_context(tc.tile_pool(name="res", bufs=1))
    big_pool = ctx.enter_context(tc.tile_pool(name="big", bufs=2))
    fast_pool = ctx.enter_context(tc.tile_pool(name="fastres", bufs=1))
    exp_pool = ctx.enter_context(tc.tile_pool(name="exps", bufs=3))
    junk_pool = ctx.enter_context(tc.tile_pool(name="junks", bufs=2))

    def rtile(name, ncol=NCOL, dtype=F32):
        return res_pool.tile([P, ncol], dtype, name=name, tag=name)

    # ---- constants ----
    iota_f = const_pool.tile([P, NBINS], F32, name="iota_f", tag="iota_f")
    if level >= 2:
        iota_i = const_pool.tile([P, NBINS], I32, name="iota_i", tag="iota_i")
        nc.gpsimd.iota(iota_i[:], pattern=[[1, NBINS]], base=0, channel_multiplier=0)
        nc.vector.tensor_copy(iota_f[:], iota_i[:])
    else:
        nc.vector.memset(iota_f[:], 1.0)

    # ---- assembled per-row result tiles [P, NCOL] ----
    sum_r = rtile("sum_r")
    wsum_r = rtile("wsum_r")
    sum_s = rtile("sum_s")
    wsum_s = rtile("wsum_s")
    sum_f = rtile("sum_f")
    fdot = rtile("fdot")
    sum_a = rtile("sum_a")
    padot = rtile("padot")
    alp_raw = rtile("alp_raw")
    f_k = rtile("f_k")
    f_k1 = rtile("f_k1")

    cont_asm = rtile("cont_asm")
    actf_asm = rtile("actf_asm")

    # ---- load small tensors (already time-reversed on host) ----
    for tb in range(TB):
        nc.sync.dma_start(out=cont_asm[:, tb * T:(tb + 1) * T], in_=cont_v[tb])
        nc.sync.dma_start(out=actf_asm[:, tb * T:(tb + 1) * T], in_=actf_v[tb])

    actf1 = rtile("actf1")
    nc.vector.tensor_scalar(actf1[:], actf_asm[:], 1.0, None, Alu.add)

    fst_tiles = []

    # ================= Phase A: streaming softmax stats =================
    for tb in range(TB):
        rew_t = big_pool.tile([P, T * NBINS], F32, name=f"rew_sb{tb}", tag="rew_sb")
        nc.sync.dma_start(out=rew_t[:], in_=rew_v[tb])
        slw_t = big_pool.tile([P, T * NBINS], F32, name=f"slw_sb{tb}", tag="slw_sb")
        nc.sync.dma_start(out=slw_t[:], in_=slw_v[tb])
        fst_t = fast_pool.tile([P, T * NBINS], F32, name=f"fst_sb{tb}", tag=f"fst_sb{tb}")
        nc.sync.dma_start(out=fst_t[:], in_=fst_v[tb])
        fst_tiles.append(fst_t)
        act_t = big_pool.tile([P, T * A], F32, name=f"act_sb{tb}", tag="act_sb")
        nc.sync.dma_start(out=act_t[:], in_=actl_v[tb])

        for t in range(T):
            col = tb * T + (T - 1 - t)  # time-reversed column
            cs = slice(col, col + 1)
            r_sl = rew_t[:, t * NBINS:(t + 1) * NBINS]
            s_sl = slw_t[:, t * NBINS:(t + 1) * NBINS]
            f_sl = fst_t[:, t * NBINS:(t + 1) * NBINS]
            a_sl = act_t[:, t * A:(t + 1) * A]

            # reward decode stats
            exp_r = exp_pool.tile([P, NBINS], F32, name="exp_r", tag="exp_r")
            nc.scalar.activation(exp_r[:], r_sl, Act.Exp, accum_out=sum_r[:, cs])
            if level >= 2:
                jnk_r = junk_pool.tile([P, NBINS], F32, name="jnk_r", tag="jnk_r")
                nc.vector.affine_mul_reduce(
                    jnk_r[:], wsum_r[:, cs], iota_f[:], exp_r[:], STEP, LOW
                )
            else:
                nc.vector.tensor_reduce(
                    wsum_r[:, cs], exp_r[:], mybir.AxisListType.X, Alu.add
                )

            # slow critic decode stats + slow_probs . fast dot
            exp_s = exp_pool.tile([P, NBINS], F32, name="exp_s", tag="exp_s")
            nc.scalar.activation(exp_s[:], s_sl, Act.Exp, accum_out=sum_s[:, cs])
            if level >= 2:
                jnk_s = junk_pool.tile([P, NBINS], F32, name="jnk_s", tag="jnk_s")
                nc.vector.affine_mul_reduce(
                    jnk_s[:], wsum_s[:, cs], iota_f[:], exp_s[:], STEP, LOW
                )
            else:
                nc.vector.tensor_reduce(
                    wsum_s[:, cs], exp_s[:], mybir.AxisListType.X, Alu.add
                )
            if level >= 3:
                jnk_d = junk_pool.tile([P, NBINS], F32, name="jnk_d", tag="jnk_d")
                _ttr(nc, jnk_d[:], exp_s[:], f_sl, fdot[:, cs])
            else:
                nc.vector.tensor_reduce(
                    fdot[:, cs], exp_s[:], mybir.AxisListType.X, Alu.add
                )

            # fast critic logsumexp stats
            exp_f = exp_pool.tile([P, NBINS], F32, name="exp_f", tag="exp_f")
            nc.scalar.activation(exp_f[:], f_sl, Act.Exp, accum_out=sum_f[:, cs])

            # actions: softmax stats, entropy dot, chosen-logit gather
            exp_a = exp_pool.tile([P, A], F32, name="exp_a", tag="exp_a")
            nc.scalar.activation(exp_a[:], a_sl, Act.Exp, accum_out=sum_a[:, cs])
            if level >= 3:
                jnk_a = junk_pool.tile([P, A], F32, name="jnk_a", tag="jnk_a")
                _ttr(nc, jnk_a[:], exp_a[:], a_sl, padot[:, cs])
            else:
                nc.vector.tensor_reduce(
                    padot[:, cs], exp_a[:], mybir.AxisListType.X, Alu.add
                )
            if level >= 4:
                jnk_m = junk_pool.tile([P, A], F32, name="jnk_m", tag="jnk_m")
                _mask_gather(
                    nc, jnk_m[:], a_sl, actf_asm[:, cs], actf1[:, cs],
                    alp_raw[:, cs],
                )
            else:
                nc.vector.tensor_reduce(
                    alp_raw[:, cs], exp_a[:], mybir.AxisListType.X, Alu.add
                )

    # ================= Phase B: per-row math on [P, NCOL] =================
    def symexp_from(sumt, wsumt, outname):
        rcp = rtile("rcp_" + outname)
        nc.vector.reciprocal(rcp[:], sumt[:])
        y = rtile("y_" + outname)
        nc.vector.tensor_mul(y[:], wsumt[:], rcp[:])
        t_abs = rtile("abs_" + outname)
        nc.scalar.activation(t_abs[:], y[:], Act.Abs)
        t_exp = rtile("exp_" + outname)
        nc.scalar.activation(t_exp[:], t_abs[:], Act.Exp)
        t_sgn = rtile("sgn_" + outname)
        nc.scalar.activation(t_sgn[:], y[:], Act.Sign)
        out = rtile(outname)
        # (exp(|y|) - 1) * sign(y)
        nc.vector.scalar_tensor_tensor(
            out[:], t_exp[:], -1.0, t_sgn[:], Alu.add, Alu.mult
        )
        return out

    rewards = symexp_from(sum_r, wsum_r, "rewards")
    values = symexp_from(sum_s, wsum_s, "values")

    # continues = sigmoid(x) = 1 / (1 + exp(-x))
    c_e = rtile("c_e")
    nc.scalar.activation(c_e[:], cont_asm[:], Act.Exp, scale=-1.0)
    c_d = rtile("c_d")
    nc.vector.tensor_scalar(c_d[:], c_e[:], 1.0, None, Alu.add)
    continues = rtile("continues")
    nc.vector.reciprocal(continues[:], c_d[:])

    # lambda-return scan; columns are time-reversed so scan runs forward.
    # R[j] = r[j] + g*c[j]*((1-lam)*v_next[j] + lam*R[j-1]),
    # where v_next[j] = values[:, j-1]; R[0] = values[:, 0].
    lam_t = rtile("lam_t")
    for tb in range(TB):
        o = tb * T
        nc.vector.tensor_copy(lam_t[:, o:o + 1], values[:, o:o + 1])
        c_sl = continues[:, o + 1:o + T]
        v_nx = values[:, o:o + T - 1]
        r_sl = rewards[:, o + 1:o + T]
        u = res_pool.tile([P, T - 1], F32, name=f"scan_u{tb}", tag="scan_u")
        nc.vector.tensor_mul(u[:], c_sl, v_nx)
        b_t = res_pool.tile([P, T - 1], F32, name=f"scan_b{tb}", tag="scan_b")
        nc.vector.scalar_tensor_tensor(
            b_t[:], u[:], GAMMA * (1.0 - LAM), r_sl, Alu.mult, Alu.add
        )
        a_t = res_pool.tile([P, T - 1], F32, name=f"scan_a{tb}", tag="scan_a")
        nc.vector.tensor_scalar(a_t[:], c_sl, GAMMA * LAM, None, Alu.mult)
        if level >= 5:
            # state = (a * state) + b
            nc.vector.tensor_tensor_scan(
                lam_t[:, o + 1:o + T], a_t[:], b_t[:], values[:, o:o + 1],
                Alu.mult, Alu.add,
            )
        else:
            nc.vector.tensor_copy(lam_t[:, o + 1:o + T], b_t[:])

    # two-hot position: pos = (clip(symlog(lam), LOW, HIGH) - LOW) / STEP
    l_abs = rtile("l_abs")
    nc.scalar.activation(l_abs[:], lam_t[:], Act.Abs)
    l_log = rtile("l_log")
    nc.scalar.activation(l_log[:], l_abs[:], Act.Ln, bias=1.0, scale=1.0)  # log1p
    l_sgn = rtile("l_sgn")
    nc.scalar.activation(l_sgn[:], lam_t[:], Act.Sign)
    y2 = rtile("y2")
    nc.vector.tensor_mul(y2[:], l_log[:], l_sgn[:])
    y2c = rtile("y2c")
    nc.vector.tensor_scalar(y2c[:], y2[:], HIGH, LOW, Alu.min, Alu.max)
    pos = rtile("pos")
    nc.vector.tensor_scalar(pos[:], y2c[:], -LOW, 1.0 / STEP, Alu.add, Alu.mult)

    # k = clip(floor(pos), 0, 253) ; works whether f32->i32 rounds or truncs
    k_i = res_pool.tile([P, NCOL], I32, name="k_i", tag="k_i")
    nc.vector.tensor_copy(k_i[:], pos[:])
    k_f0 = rtile("k_f0")
    nc.vector.tensor_copy(k_f0[:], k_i[:])
    k_gt = rtile("k_gt")
    nc.vector.tensor_tensor(k_gt[:], k_f0[:], pos[:], Alu.is_gt)
    k_fl = rtile("k_fl")
    nc.vector.tensor_sub(k_fl[:], k_f0[:], k_gt[:])
    kf = rtile("kf")
    nc.vector.tensor_scalar(kf[:], k_fl[:], 0.0, float(NBINS - 2), Alu.max, Alu.min)
    w_t = rtile("w_t")
    nc.vector.tensor_sub(w_t[:], pos[:], kf[:])
    kf1 = rtile("kf1")
    nc.vector.tensor_scalar(kf1[:], kf[:], 1.0, None, Alu.add)
    kf2 = rtile("kf2")
    nc.vector.tensor_scalar(kf2[:], kf[:], 2.0, None, Alu.add)

    # gather fast logits at k and k+1 (width-1 max-masks)
    for tb in range(TB):
        fst_t = fst_tiles[tb]
        for t in range(T):
            col = tb * T + (T - 1 - t)
            cs = slice(col, col + 1)
            f_sl = fst_t[:, t * NBINS:(t + 1) * NBINS]
            if level >= 4:
                jnk_g = junk_pool.tile([P, NBINS], F32, name="jnk_g", tag="jnk_g")
                _mask_gather(
                    nc, jnk_g[:], f_sl, kf[:, cs], kf1[:, cs], f_k[:, cs]
                )
                jnk_h = junk_pool.tile([P, NBINS], F32, name="jnk_h", tag="jnk_h")
                _mask_gather(
                    nc, jnk_h[:], f_sl, kf1[:, cs], kf2[:, cs], f_k1[:, cs]
                )
            else:
                nc.vector.tensor_reduce(
                    f_k[:, cs], f_sl, mybir.AxisListType.X, Alu.max
                )
                nc.vector.tensor_reduce(
                    f_k1[:, cs], f_sl, mybir.AxisListType.X, Alu.max
                )

    # ================= Phase C: final row-space terms + partial sums =======
    # entropy = lse_a - padot / sum_a ; alp = alp_raw - lse_a
    rcp_a = rtile("rcp_a")
    nc.vector.reciprocal(rcp_a[:], sum_a[:])
    pd_n = rtile("pd_n")
    nc.vector.tensor_mul(pd_n[:], padot[:], rcp_a[:])
    lse_a = rtile("lse_a")
    nc.scalar.activation(lse_a[:], sum_a[:], Act.Ln)
    ent = rtile("ent")
    nc.vector.tensor_sub(ent[:], lse_a[:], pd_n[:])
    alp = rtile("alp")
    nc.vector.tensor_sub(alp[:], alp_raw[:], lse_a[:])

    lse_f = rtile("lse_f")
    nc.scalar.activation(lse_f[:], sum_f[:], Act.Ln)

    # g = f_k + w * (f_k1 - f_k)
    g_d = rtile("g_d")
    nc.vector.tensor_sub(g_d[:], f_k1[:], f_k[:])
    g_m = rtile("g_m")
    nc.vector.tensor_mul(g_m[:], w_t[:], g_d[:])
    g_t = rtile("g_t")
    nc.vector.tensor_add(g_t[:], f_k[:], g_m[:])

    # advantage = lam - values
    adv = rtile("adv")
    nc.vector.tensor_sub(adv[:], lam_t[:], values[:])

    # fdot normalized by sum_s
    rcp_s = rtile("rcp_s")
    nc.vector.reciprocal(rcp_s[:], sum_s[:])
    fdn = rtile("fdn")
    nc.vector.tensor_mul(fdn[:], fdot[:], rcp_s[:])

    parts = res_pool.tile([P, 8], F32, name="parts", tag="parts")
    jnk_p = rtile("jnk_p")
    nc.vector.scalar_tensor_tensor(
        jnk_p[:], adv[:], 1.0, alp[:], Alu.mult, Alu.mult,
        accum_out=parts[:, 0:1],
    )
    nc.vector.tensor_reduce(parts[:, 1:2], ent[:], mybir.AxisListType.X, Alu.add)
    nc.vector.tensor_reduce(parts[:, 2:3], lse_f[:], mybir.AxisListType.X, Alu.add)
    nc.vector.tensor_reduce(parts[:, 3:4], g_t[:], mybir.AxisListType.X, Alu.add)
    nc.vector.tensor_reduce(parts[:, 4:5], fdn[:], mybir.AxisListType.X, Alu.add)
    nc.vector.memset(parts[:, 5:8], 0.0)

    # ---- outputs ----
    for tb in range(TB):
        nc.sync.dma_start(out=lam_v[tb], in_=lam_t[:, tb * T:(tb + 1) * T])
    nc.sync.dma_start(out=parts_out[:], in_=parts[:])

    ctx.close()


def _install_ntff_hook_shim():
    """This image's `antenv` lacks `axon_hooks`; replicate the boot-time
    NTFF profile hook (ctypes into libaxon_pjrt.so) so trace=True works."""
    try:
        from antenv.axon_hooks import get_axon_ntff_profile_hook  # noqa: F401

        return
    except ImportError:
        pass
    import contextlib
    import ctypes
    import types

    so_path = "/opt/axon/libaxon_pjrt.so"
    hook = None
    try:
        lib = ctypes.CDLL(so_path)
        if hasattr(lib, "axon_start_nrt_profile"):
            lib.axon_start_nrt_profile.argtypes = [
                ctypes.POINTER(ctypes.c_int64),
                ctypes.c_size_t,
            ]
            lib.axon_start_nrt_profile.restype = ctypes.c_int64
            lib.axon_stop_nrt_profile.argtypes = [ctypes.c_char_p]
            lib.axon_stop_nrt_profile.restype = ctypes.c_int64

            @contextlib.contextmanager
            def _hook(output_dir, device_ids):
                import jax

                jax.devices()
                if device_ids:
                    ids = (ctypes.c_int64 * len(device_ids))(*device_ids)
                    rc = lib.axon_start_nrt_profile(ids, len(device_ids))
                else:
                    rc = lib.axon_start_nrt_profile(None, 0)
                if rc != 0:
                    raise RuntimeError(f"axon_start_nrt_profile rc={rc}")
                try:
                    yield
                finally:
                    n = lib.axon_stop_nrt_profile(str(output_dir).encode())
                    if n < 0:
                        raise RuntimeError(f"axon_stop_nrt_profile rc={n}")
                    print(f"profile: {n} file(s) written to {output_dir}")

            hook = _hook
    except OSError:
        pass

    mod = types.ModuleType("antenv.axon_hooks")
    mod._hook = hook
    mod.get_axon_ntff_profile_hook = lambda: mod._hook
    mod.set_axon_ntff_profile_hook = lambda h: setattr(mod, "_hook", h)
    sys.modules["antenv.axon_hooks"] = mod


_CACHE = {}


def _get_compiled(level: int = 99):
    key = ("nc", level)
    if key not in _CACHE:
        nc = bacc.Bacc(
            "TRN2", target_bir_lowering=False, debug=False, num_devices=NCORES
        )
        with tile.TileContext(nc) as tc:
            build_kernel(nc, tc, level=level)
        nc.compile()
        _CACHE[key] = nc
    return _CACHE[key]


def _make_in_maps(inputs):
    rew = np.ascontiguousarray(
        np.asarray(inputs["predicted_reward_logits"], dtype=np.float32)
    )
    slw = np.ascontiguousarray(
        np.asarray(inputs["slow_critic_logits"], dtype=np.float32)
    )
    fst = np.ascontiguousarray(
        np.asarray(inputs["fast_critic_logits"], dtype=np.float32)
    )
    actl = np.ascontiguousarray(np.asarray(inputs["action_logits"], dtype=np.float32))
    cont = np.asarray(inputs["predicted_continue_logits"], dtype=np.float32)[..., 0]
    actf = np.asarray(inputs["actions"]).astype(np.float32)
    # pre-reverse T for the tensors that are DMA'd straight into the
    # time-reversed assembled column layout
    cont_r = np.ascontiguousarray(cont[:, ::-1])
    actf_r = np.ascontiguousarray(actf[:, ::-1])

    in_maps = []
    for i in range(NCORES):
        s = slice(i * BS, (i + 1) * BS)
        in_maps.append(
            {
                "rew": rew[s],
                "slw": slw[s],
                "fst": fst[s],
                "actl": actl[s],
                "cont": cont_r[s],
                "actf": actf_r[s],
            }
        )
    return in_maps


def _combine(results):
    lam_all = np.concatenate(
        [np.asarray(r["lam_out"], dtype=np.float64).reshape(-1) for r in results]
    )
    S = np.zeros(5, dtype=np.float64)
    for r in results:
        S += np.asarray(r["parts_out"], dtype=np.float64)[:, :5].sum(axis=0)
    n = float(B * T)
    p_hi = np.quantile(lam_all, 0.95)
    p_lo = np.quantile(lam_all, 0.05)
    norm = max(p_hi - p_lo, 1.0)
    actor = -S[0] / (n * norm) - ENT_COEF * S[1] / n
    critic = (S[2] - S[3]) / n + SLOW_W * (S[2] - S[4]) / n
    return np.float32(actor + critic)


def run(inputs, trace=False, level: int = 99, **kw):
    if trace:
        _install_ntff_hook_shim()
    nc = _get_compiled(level)
    in_maps = _make_in_maps(inputs)
    res = bass_utils.run_bass_kernel_spmd(
        nc, in_maps, core_ids=list(range(NCORES)), trace=trace, **kw
    )
    return _combine(res.results), res


def kernel(**inputs) -> np.ndarray:
    out, _ = run(inputs)
    return out


# revision 24
# speedup vs baseline: 1.5039x; 1.5039x over previous
"""Trainium2 Bass kernel for the DreamerV3-style ActorCriticLoss.

Contract: kernel(**inputs) takes the FULL (unsharded) numpy inputs and
returns the FULL output (a float32 scalar loss). Internally the batch dim
(B=4096) is sharded 8 ways (pure data parallel); each NeuronCore computes
everything except the two lambda-return quantiles and the final scalar
combine, which run on host after the gather (per-row work is all on
device; host only sums per-partition partials, takes the quantiles of the
device-computed lambda returns, and assembles the scalar).

Self-contained: hardcodes shapes from the problem spec.
"""

import sys
from contextlib import ExitStack

sys.path.insert(0, "/opt/trn_rl_repo")

import numpy as np

import concourse.bass as bass  # noqa: E402
import concourse.bacc as bacc  # noqa: E402
import concourse.mybir as mybir  # noqa: E402
from concourse import bass_utils  # noqa: E402
from concourse import tile  # noqa: E402

# ---- problem constants (from reference.py) ----
LOW, HIGH, NBINS = -20.0, 20.0, 255
GAMMA, LAM = 0.99, 0.95
ENT_COEF, SLOW_W = 0.05, 1.0
STEP = (HIGH - LOW) / (NBINS - 1)
B, T, A = 4096, 16, 32

NCORES = 8
BS = B // NCORES  # 512 batch rows per core
P = 128  # partitions
TB = BS // P  # 4 partition-blocks per core
NCOL = TB * T  # 64 columns in the assembled per-row tiles

F32 = mybir.dt.float32
I32 = mybir.dt.int32
Alu = mybir.AluOpType
Act = mybir.ActivationFunctionType

NEG_BIG = -3.0e38


_TWOHOT_OP = None


def _register_twohot_op():
    """Author + register a fused custom-DVE op at runtime:
        body  = relu(C1 - |Idx - C0|) * Src0
        accum = sum(body)
    With C0 = pos (per-partition) and C1 = 1.0 this computes the exact
    two-hot interpolation  (1-w)*x[k] + w*x[k+1]  in a single pass
    (the triangular hat places 1-w on floor(pos) and w on floor(pos)+1).
    """
    global _TWOHOT_OP
    if _TWOHOT_OP is not None:
        return _TWOHOT_OP
    import numpy as np
    from operator import add as _add

    from concourse import dve_ops
    from concourse.dve_spec import (
        C0,
        C1,
        Idx,
        Spec,
        Zero,
        lower,
        maxx,
        relu,
        _has_src1,
    )
    from concourse.dve_uop import DveOpSpec

    name = "TWOHOT_DOT_ANT"
    for op in dve_ops.OPS:
        if op.name == name:
            _TWOHOT_OP = op
            return op

    d = Idx - C0
    body = relu(C1 - maxx(d, Zero - d)) * Src0_leaf()

    def ref(in0, in1, c0, c1, c2):
        n = in0.shape[-1]
        idx = np.arange(n, dtype=np.float32)
        if isinstance(c0, np.ndarray):
            c0 = c0.reshape(-1, *([1] * (in0.ndim - 1)))
        hat = np.maximum(
            np.float32(c1) - np.abs(idx.reshape(*([1] * (in0.ndim - 1)), n) - c0),
            0.0,
        )
        b = (hat * in0.astype(np.float32)).astype(np.float32)
        return b, b.reshape(b.shape[0], -1).sum(axis=-1, keepdims=True)

    spec = Spec(body=body, accum=_add, accum_init=Zero, reference=ref)
    row = max(dve_ops._SUB_OPCODE_FOR_NAME.values()) + 1
    assert row < 0x20
    dve_ops._SUB_OPCODE_FOR_NAME[name] = row
    # compute the sha pins by lowering for both vers
    shas = {}
    for ver in ("v3", "v4"):
        try:
            s = DveOpSpec(
                name=name, opcode=row, uops=lower(spec, ver=ver),
                rd1_en=_has_src1(spec),
            )
            shas[ver] = s.sha(ver)
        except Exception:
            pass
    op = dve_ops.DveOp(name, spec, subdim=False, uops_sha=shas)
    dve_ops.OPS.append(op)
    dve_ops.CUSTOM_DVE_SPECS[name] = spec
    _TWOHOT_OP = op
    return op


def Src0_leaf():
    from concourse.dve_spec import Src0

    return Src0


def _twohot(nc, out, data, pos_col, accum_out):
    op = _register_twohot_op()
    nc.vector._custom_dve(
        op, out=out, in0=data, s0=pos_col, s1=1.0, accum_out=accum_out
    )


def _ttr(nc, out, in0, in1, accum_out):
    """(in0*in1) elementwise with accum_out = sum — via the production
    custom-DVE op (the TENSOR_TENSOR_REDUCE ISA opcode crashes at runtime
    on this stack; the custom-DVE table path works)."""
    from concourse.dve_ops import TENSOR_TENSOR_REDUCE as _OP

    nc.vector._custom_dve(
        _OP, out=out, in0=in0, in1=in1, s0=0.0, s1=1.0, accum_out=accum_out
    )


def _mask_gather(nc, out, data, start_col, end_col, accum_out):
    """accum_out = max over idx in [start, end) of data — the width-1 range
    makes this an exact per-partition dynamic gather. Custom-DVE variant
    (the TENSOR_MASK_REDUCE ISA opcode crashes at runtime on this stack).
    C0=s0=start, C3=in1=end (spilled), C1=s1=accum seed, C2=imm2=scale."""
    from concourse.dve_ops import TENSOR_MASK_REDUCE as _OP

    nc.vector._custom_dve(
        _OP,
        out=out,
        in0=data,
        in1=end_col,
        s0=start_col,
        s1=NEG_BIG,
        imm2=1.0,
        accum_out=accum_out,
    )


def build_kernel(nc: bass.Bass, tc: "tile.TileContext", level: int = 99):
    """Per-core program. ALL inputs arrive with the T axis REVERSED on the
    host (zero-copy views; PJRT staging makes them contiguous), so column
    j = T-1-t everywhere and the lambda-return scan runs FORWARD along the
    free dim. lam_out column order is irrelevant on host (quantiles)."""

    # ---- DRAM I/O ----
    rew_d = nc.dram_tensor("rew", [BS, T, NBINS], F32, kind="ExternalInput").ap()
    slw_d = nc.dram_tensor("slw", [BS, T, NBINS], F32, kind="ExternalInput").ap()
    fst_d = nc.dram_tensor("fst", [BS, T, NBINS], F32, kind="ExternalInput").ap()
    actl_d = nc.dram_tensor("actl", [BS, T, A], F32, kind="ExternalInput").ap()
    cont_d = nc.dram_tensor("cont", [BS, T], F32, kind="ExternalInput").ap()
    actf_d = nc.dram_tensor("actf", [BS, T], F32, kind="ExternalInput").ap()

    lam_out = nc.dram_tensor("lam_out", [BS, T], F32, kind="ExternalOutput").ap()
    parts_out = nc.dram_tensor("parts_out", [P, 8], F32, kind="ExternalOutput").ap()

    rew_v = rew_d.rearrange("(tb p) t n -> tb p (t n)", p=P)
    slw_v = slw_d.rearrange("(tb p) t n -> tb p (t n)", p=P)
    fst_v = fst_d.rearrange("(tb p) t n -> tb p (t n)", p=P)
    actl_v = actl_d.rearrange("(tb p) t a -> tb p (t a)", p=P)
    cont_v = cont_d.rearrange("(tb p) t -> tb p t", p=P)
    actf_v = actf_d.rearrange("(tb p) t -> tb p t", p=P)
    lam_v = lam_out.rearrange("(tb p) t -> tb p t", p=P)

    ctx = ExitStack()
    const_pool = ctx.enter_context(tc.tile_pool(name="const", bufs=1))
    res_pool = ctx.enter_context(tc.tile_pool(name="res", bufs=1))
    big_pool = ctx.enter_context(tc.tile_pool(name="big", bufs=2))
    fast_pool = ctx.enter_context(tc.tile_pool(name="fastres", bufs=1))
    exp_pool = ctx.enter_context(tc.tile_pool(name="exps", bufs=3))
    junk_pool = ctx.enter_context(tc.tile_pool(name="junks", bufs=2))

    def rtile(name, ncol=NCOL, dtype=F32):
        return res_pool.tile([P, ncol], dtype, name=name, tag=name)

    # ---- constants ----
    iota_i = const_pool.tile([P, NBINS], I32, name="iota_i", tag="iota_i")
    nc.gpsimd.iota(iota_i[:], pattern=[[1, NBINS]], base=0, channel_multiplier=0)
    iota_f = const_pool.tile([P, NBINS], F32, name="iota_f", tag="iota_f")
    nc.vector.tensor_copy(iota_f[:], iota_i[:])

    # ---- assembled per-row result tiles [P, NCOL] (all in j = T-1-t order) --
    sum_r = rtile("sum_r")
    wsum_r = rtile("wsum_r")
    sum_s = rtile("sum_s")
    wsum_s = rtile("wsum_s")
    sum_f = rtile("sum_f")
    fdot = rtile("fdot")
    sum_a = rtile("sum_a")
    padot = rtile("padot")
    alp_raw = rtile("alp_raw")
    g_t = rtile("g_t")

    cont_asm = rtile("cont_asm")
    actf_asm = rtile("actf_asm")

    for tb in range(TB):
        nc.sync.dma_start(out=cont_asm[:, tb * T:(tb + 1) * T], in_=cont_v[tb])
        nc.sync.dma_start(out=actf_asm[:, tb * T:(tb + 1) * T], in_=actf_v[tb])

    fst_tiles = []

    # ================= Phase A: streaming softmax stats =================
    for tb in range(TB):
        o = tb * T
        rew_t = big_pool.tile([P, T * NBINS], F32, name=f"rew_sb{tb}", tag="rew_sb")
        nc.sync.dma_start(out=rew_t[:], in_=rew_v[tb])
        slw_t = big_pool.tile([P, T * NBINS], F32, name=f"slw_sb{tb}", tag="slw_sb")
        nc.sync.dma_start(out=slw_t[:], in_=slw_v[tb])
        fst_t = fast_pool.tile([P, T * NBINS], F32, name=f"fst_sb{tb}", tag=f"fst_sb{tb}")
        nc.sync.dma_start(out=fst_t[:], in_=fst_v[tb])
        fst_tiles.append(fst_t)
        act_t = big_pool.tile([P, T * A], F32, name=f"act_sb{tb}", tag="act_sb")
        nc.sync.dma_start(out=act_t[:], in_=actl_v[tb])

        # ---- batched action stats: one wide exp + 3D axis-X reduces ----
        exp_a_full = big_pool.tile([P, T * A], F32, name=f"exp_a{tb}", tag="exp_a_f")
        nc.scalar.activation(exp_a_full[:], act_t[:], Act.Exp)
        nc.vector.tensor_reduce(
            sum_a[:, o:o + T],
            exp_a_full[:].rearrange("p (t a) -> p t a", a=A),
            mybir.AxisListType.X,
            Alu.add,
        )
        prod_a = big_pool.tile([P, T * A], F32, name=f"prod_a{tb}", tag="prod_a")
        nc.vector.tensor_mul(prod_a[:], exp_a_full[:], act_t[:])
        nc.vector.tensor_reduce(
            padot[:, o:o + T],
            prod_a[:].rearrange("p (t a) -> p t a", a=A),
            mybir.AxisListType.X,
            Alu.add,
        )

        for t in range(T):
            col = o + t
            cs = slice(col, col + 1)
            r_sl = rew_t[:, t * NBINS:(t + 1) * NBINS]
            s_sl = slw_t[:, t * NBINS:(t + 1) * NBINS]
            f_sl = fst_t[:, t * NBINS:(t + 1) * NBINS]
            a_sl = act_t[:, t * A:(t + 1) * A]

            # reward decode stats
            exp_r = exp_pool.tile([P, NBINS], F32, name="exp_r", tag="exp_r")
            nc.scalar.activation(exp_r[:], r_sl, Act.Exp, accum_out=sum_r[:, cs])
            jnk_r = junk_pool.tile([P, NBINS], F32, name="jnk_r", tag="jnk_r")
            nc.vector.affine_mul_reduce(
                jnk_r[:], wsum_r[:, cs], iota_f[:], exp_r[:], STEP, LOW
            )

            # slow critic decode stats + slow_probs . fast dot
            exp_s = exp_pool.tile([P, NBINS], F32, name="exp_s", tag="exp_s")
            nc.scalar.activation(exp_s[:], s_sl, Act.Exp, accum_out=sum_s[:, cs])
            jnk_s = junk_pool.tile([P, NBINS], F32, name="jnk_s", tag="jnk_s")
            nc.vector.affine_mul_reduce(
                jnk_s[:], wsum_s[:, cs], iota_f[:], exp_s[:], STEP, LOW
            )
            jnk_d = junk_pool.tile([P, NBINS], F32, name="jnk_d", tag="jnk_d")
            _ttr(nc, jnk_d[:], exp_s[:], f_sl, fdot[:, cs])

            # fast critic logsumexp stats
            exp_f = exp_pool.tile([P, NBINS], F32, name="exp_f", tag="exp_f")
            nc.scalar.activation(exp_f[:], f_sl, Act.Exp, accum_out=sum_f[:, cs])

            # chosen-action logit gather: integer pos -> exact one-hot
            jnk_m = junk_pool.tile([P, A], F32, name="jnk_m", tag="jnk_m")
            _twohot(nc, jnk_m[:], a_sl, actf_asm[:, cs], alp_raw[:, cs])

    # ================= Phase B: per-row math on [P, NCOL] =================
    def symexp_from(sumt, wsumt, outname):
        rcp = rtile("rcp_" + outname)
        nc.vector.reciprocal(rcp[:], sumt[:])
        y = rtile("y_" + outname)
        nc.vector.tensor_mul(y[:], wsumt[:], rcp[:])
        t_abs = rtile("abs_" + outname)
        nc.scalar.activation(t_abs[:], y[:], Act.Abs)
        t_exp = rtile("exp_" + outname)
        nc.scalar.activation(t_exp[:], t_abs[:], Act.Exp)
        t_sgn = rtile("sgn_" + outname)
        nc.scalar.activation(t_sgn[:], y[:], Act.Sign)
        out = rtile(outname)
        # (exp(|y|) - 1) * sign(y)
        nc.vector.scalar_tensor_tensor(
            out[:], t_exp[:], -1.0, t_sgn[:], Alu.add, Alu.mult
        )
        return out

    rewards = symexp_from(sum_r, wsum_r, "rewards")
    values = symexp_from(sum_s, wsum_s, "values")

    # continues = sigmoid(x) = 1 / (1 + exp(-x))
    c_e = rtile("c_e")
    nc.scalar.activation(c_e[:], cont_asm[:], Act.Exp, scale=-1.0)
    c_d = rtile("c_d")
    nc.vector.tensor_scalar(c_d[:], c_e[:], 1.0, None, Alu.add)
    continues = rtile("continues")
    nc.vector.reciprocal(continues[:], c_d[:])

    # lambda-return scan; columns are time-reversed so scan runs forward.
    # R[j] = r[j] + g*c[j]*((1-lam)*v_next[j] + lam*R[j-1]),
    # where v_next[j] = values[:, j-1]; R[0] = values[:, 0].
    lam_t = rtile("lam_t")
    for tb in range(TB):
        o = tb * T
        nc.vector.tensor_copy(lam_t[:, o:o + 1], values[:, o:o + 1])
        c_sl = continues[:, o + 1:o + T]
        v_nx = values[:, o:o + T - 1]
        r_sl = rewards[:, o + 1:o + T]
        u = res_pool.tile([P, T - 1], F32, name=f"scan_u{tb}", tag="scan_u")
        nc.vector.tensor_mul(u[:], c_sl, v_nx)
        b_t = res_pool.tile([P, T - 1], F32, name=f"scan_b{tb}", tag="scan_b")
        nc.vector.scalar_tensor_tensor(
            b_t[:], u[:], GAMMA * (1.0 - LAM), r_sl, Alu.mult, Alu.add
        )
        a_t = res_pool.tile([P, T - 1], F32, name=f"scan_a{tb}", tag="scan_a")
        nc.vector.tensor_scalar(a_t[:], c_sl, GAMMA * LAM, None, Alu.mult)
        # state = (a * state) + b
        nc.vector.tensor_tensor_scan(
            lam_t[:, o + 1:o + T], a_t[:], b_t[:], values[:, o:o + 1],
            Alu.mult, Alu.add,
        )

    # two-hot position: pos = (clip(symlog(lam), LOW, HIGH) - LOW) / STEP
    l_abs = rtile("l_abs")
    nc.scalar.activation(l_abs[:], lam_t[:], Act.Abs)
    l_log = rtile("l_log")
    nc.scalar.activation(l_log[:], l_abs[:], Act.Ln, bias=1.0, scale=1.0)  # log1p
    l_sgn = rtile("l_sgn")
    nc.scalar.activation(l_sgn[:], lam_t[:], Act.Sign)
    y2 = rtile("y2")
    nc.vector.tensor_mul(y2[:], l_log[:], l_sgn[:])
    y2c = rtile("y2c")
    nc.vector.tensor_scalar(y2c[:], y2[:], HIGH, LOW, Alu.min, Alu.max)
    pos = rtile("pos")
    nc.vector.tensor_scalar(pos[:], y2c[:], -LOW, 1.0 / STEP, Alu.add, Alu.mult)

    # fused two-hot CE dot: g = (1-w)*fst[k] + w*fst[k+1] in ONE pass per tile
    for tb in range(TB):
        fst_t = fst_tiles[tb]
        for t in range(T):
            col = tb * T + t
            cs = slice(col, col + 1)
            f_sl = fst_t[:, t * NBINS:(t + 1) * NBINS]
            jnk_g = junk_pool.tile([P, NBINS], F32, name="jnk_g", tag="jnk_g")
            _twohot(nc, jnk_g[:], f_sl, pos[:, cs], g_t[:, cs])

    # ================= Phase C: final row-space terms + partial sums =======
    # entropy = lse_a - padot / sum_a ; alp = alp_raw - lse_a
    rcp_a = rtile("rcp_a")
    nc.vector.reciprocal(rcp_a[:], sum_a[:])
    pd_n = rtile("pd_n")
    nc.vector.tensor_mul(pd_n[:], padot[:], rcp_a[:])
    lse_a = rtile("lse_a")
    nc.scalar.activation(lse_a[:], sum_a[:], Act.Ln)
    ent = rtile("ent")
    nc.vector.tensor_sub(ent[:], lse_a[:], pd_n[:])
    alp = rtile("alp")
    nc.vector.tensor_sub(alp[:], alp_raw[:], lse_a[:])

    lse_f = rtile("lse_f")
    nc.scalar.activation(lse_f[:], sum_f[:], Act.Ln)

    # advantage = lam - values
    adv = rtile("adv")
    nc.vector.tensor_sub(adv[:], lam_t[:], values[:])

    # fdot normalized by sum_s
    rcp_s = rtile("rcp_s")
    nc.vector.reciprocal(rcp_s[:], sum_s[:])
    fdn = rtile("fdn")
    nc.vector.tensor_mul(fdn[:], fdot[:], rcp_s[:])

    parts = res_pool.tile([P, 8], F32, name="parts", tag="parts")
    jnk_p = rtile("jnk_p")
    nc.vector.scalar_tensor_tensor(
        jnk_p[:], adv[:], 1.0, alp[:], Alu.mult, Alu.mult,
        accum_out=parts[:, 0:1],
    )
    nc.vector.tensor_reduce(parts[:, 1:2], ent[:], mybir.AxisListType.X, Alu.add)
    nc.vector.tensor_reduce(parts[:, 2:3], lse_f[:], mybir.AxisListType.X, Alu.add)
    nc.vector.tensor_reduce(parts[:, 3:4], g_t[:], mybir.AxisListType.X, Alu.add)
    nc.vector.tensor_reduce(parts[:, 4:5], fdn[:], mybir.AxisListType.X, Alu.add)
    nc.vector.memset(parts[:, 5:8], 0.0)

    # ---- outputs ----
    for tb in range(TB):
        nc.sync.dma_start(out=lam_v[tb], in_=lam_t[:, tb * T:(tb + 1) * T])
    nc.sync.dma_start(out=parts_out[:], in_=parts[:])

    ctx.close()


def _install_ntff_hook_shim():
    """This image's `antenv` lacks `axon_hooks`; replicate the boot-time
    NTFF profile hook (ctypes into libaxon_pjrt.so) so trace=True works."""
    try:
        from antenv.axon_hooks import get_axon_ntff_profile_hook  # noqa: F401

        return
    except ImportError:
        pass
    import contextlib
    import ctypes
    import types

    so_path = "/opt/axon/libaxon_pjrt.so"
    hook = None
    try:
        lib = ctypes.CDLL(so_path)
        if hasattr(lib, "axon_start_nrt_profile"):
            lib.axon_start_nrt_profile.argtypes = [
                ctypes.POINTER(ctypes.c_int64),
                ctypes.c_size_t,
            ]
            lib.axon_start_nrt_profile.restype = ctypes.c_int64
            lib.axon_stop_nrt_profile.argtypes = [ctypes.c_char_p]
            lib.axon_stop_nrt_profile.restype = ctypes.c_int64

            @contextlib.contextmanager
            def _hook(output_dir, device_ids):
                import jax

                jax.devices()
                if device_ids:
                    ids = (ctypes.c_int64 * len(device_ids))(*device_ids)
                    rc = lib.axon_start_nrt_profile(ids, len(device_ids))
                else:
                    rc = lib.axon_start_nrt_profile(None, 0)
                if rc != 0:
                    raise RuntimeError(f"axon_start_nrt_profile rc={rc}")
                try:
                    yield
                finally:
                    n = lib.axon_stop_nrt_profile(str(output_dir).encode())
                    if n < 0:
                        raise RuntimeError(f"axon_stop_nrt_profile rc={n}")
                    print(f"profile: {n} file(s) written to {output_dir}")

            hook = _hook
    except OSError:
        pass

    mod = types.ModuleType("antenv.axon_hooks")
    mod._hook = hook
    mod.get_axon_ntff_profile_hook = lambda: mod._hook
    mod.set_axon_ntff_profile_hook = lambda h: setattr(mod, "_hook", h)
    sys.modules["antenv.axon_hooks"] = mod


_CACHE = {}


def _get_compiled(level: int = 99):
    key = ("nc", level)
    if key not in _CACHE:
        nc = bacc.Bacc(
            "TRN2", target_bir_lowering=False, debug=False, num_devices=NCORES
        )
        with tile.TileContext(nc) as tc:
            build_kernel(nc, tc, level=level)
        nc.compile()
        _CACHE[key] = nc
    return _CACHE[key]


def _make_in_maps(inputs):
    # ALL tensors are passed time-REVERSED (views — PJRT staging copies
    # them to contiguous anyway), so the kernel's column j = T-1-t.
    rew = np.asarray(inputs["predicted_reward_logits"], dtype=np.float32)[:, ::-1]
    slw = np.asarray(inputs["slow_critic_logits"], dtype=np.float32)[:, ::-1]
    fst = np.asarray(inputs["fast_critic_logits"], dtype=np.float32)[:, ::-1]
    actl = np.asarray(inputs["action_logits"], dtype=np.float32)[:, ::-1]
    cont = np.asarray(inputs["predicted_continue_logits"], dtype=np.float32)[
        :, ::-1, 0
    ]
    actf = np.asarray(inputs["actions"]).astype(np.float32)[:, ::-1]

    in_maps = []
    for i in range(NCORES):
        s = slice(i * BS, (i + 1) * BS)
        in_maps.append(
            {
                "rew": rew[s],
                "slw": slw[s],
                "fst": fst[s],
                "actl": actl[s],
                "cont": cont[s],
                "actf": actf[s],
            }
        )
    return in_maps


def _combine(results):
    lam_all = np.concatenate(
        [np.asarray(r["lam_out"], dtype=np.float64).reshape(-1) for r in results]
    )
    S = np.zeros(5, dtype=np.float64)
    for r in results:
        S += np.asarray(r["parts_out"], dtype=np.float64)[:, :5].sum(axis=0)
    n = float(B * T)
    p_hi = np.quantile(lam_all, 0.95)
    p_lo = np.quantile(lam_all, 0.05)
    norm = max(p_hi - p_lo, 1.0)
    actor = -S[0] / (n * norm) - ENT_COEF * S[1] / n
    critic = (S[2] - S[3]) / n + SLOW_W * (S[2] - S[4]) / n
    return np.float32(actor + critic)


def run(inputs, trace=False, level: int = 99, **kw):
    if trace:
        _install_ntff_hook_shim()
    nc = _get_compiled(level)
    in_maps = _make_in_maps(inputs)
    res = bass_utils.run_bass_kernel_spmd(
        nc, in_maps, core_ids=list(range(NCORES)), trace=trace, **kw
    )
    return _combine(res.results), res


def kernel(**inputs) -> np.ndarray:
    out, _ = run(inputs)
    return out
